# revision 1
# baseline (speedup 1.0000x reference)
"""Trainium2 Bass kernel for nn_DeterministicEgnnPolicy (EGNN message passing).

Strategy (per sharding hint): shard the 1024 independent 32-node graphs
across 8 NeuronCores (128 graphs/core). On each core the fully-connected
edge structure is computed densely as all-pairs 32x32 blocks:

- "feature-major" edge tensors [128 = 2 graph-halves x 64 features,
  (gm2, i, j)] drive the edge-MLP matmuls with block-diagonal weights
  (fp32r, full 128-partition contraction).
- per-edge scalars (radial, 1/(1+sqrt(r)), u = t*s, aggregations) live in a
  "matrix" layout [128 = (g%4, i), 32x32 = (g//4, j)], 64x cheaper for
  DVE/ACT; small SBUF->SBUF DMAs convert between the layouts.
- silu+bias+PSUM-evacuation fuse into single ACT instructions.

Graph indexing on a core: g = gb*4 + gm, gb in [0,32), gm in [0,4).
half = gb//16 (feature partitions 64*half..64*half+63).
node free index (per half): n' = gb_l*128 + gm*32 + i, gb_l = gb%16.
global node: n = gb*128 + gm*32 + i.
"""

import numpy as np

N_AGENTS = 32
BATCH = 1024
H = 64
L = 4
INV = 16
DEG = float(N_AGENTS - 1)
NCORES = 8
G_CORE = BATCH // NCORES          # 128 graphs per core
NGB = G_CORE // 4                 # 32 gb blocks per core
NGBL = NGB // 2                   # 16 per half
NNODE = NGBL * 128                # 2048 node free dim (per half)
NODES_CORE = G_CORE * N_AGENTS    # 4096

_BUILD_CACHE = {}


# ----------------------------------------------------------------------------
# Host-side packing (pure layout permutation / weight arrangement)
# ----------------------------------------------------------------------------

def _bd(w):
    """64x64 block-diagonal lhsT [128,128] from w [64,64] (or [k,64])."""
    k = w.shape[0]
    out = np.zeros((128, 128), np.float32)
    out[0:k, 0:64] = w
    out[64:64 + k, 64:128] = w
    return out


def _bd_rep(wcol):
    """Replicating lhsT: out[64h+f, 64h+f'] = wcol[f] for all f'."""
    out = np.zeros((128, 128), np.float32)
    col = wcol.reshape(64, 1)
    out[0:64, 0:64] = np.repeat(col, 64, axis=1)
    out[64:128, 64:128] = np.repeat(col, 64, axis=1)
    return out


def _pack_weights(inp):
    """Build wpack [128, NW*128] and biaspack [128, NBIAS]."""
    tiles = []
    names = []

    def add(name, arr):
        t = np.zeros((128, 128), np.float32)
        t[:arr.shape[0], :arr.shape[1]] = arr
        tiles.append(t)
        names.append(name)

    add("ident", np.eye(128, dtype=np.float32))
    delta = np.zeros((4, 128), np.float32)
    for gm in range(4):
        delta[gm, gm * 32:(gm + 1) * 32] = 1.0
    add("delta", delta)

    emb = np.zeros((128, 128), np.float32)
    emb[0:INV, 0:64] = inp["emb_W"]
    emb[64:64 + INV, 64:128] = inp["emb_W"]
    add("emb", emb)

    for l in range(L):
        We1 = inp["We1"][l]          # [130, 64]
        add(f"Wi{l}", _bd(We1[0:64]))
        add(f"Wj{l}", _bd(We1[64:128]))
        wsc = np.zeros((4, 128), np.float32)
        wsc[0, 0:64] = We1[128]      # radial, half0
        wsc[1, 0:64] = We1[129]      # edge_attr, half0
        wsc[2, 64:128] = We1[128]
        wsc[3, 64:128] = We1[129]
        add(f"Wsc{l}", wsc)
        add(f"We2{l}", _bd(inp["We2"][l]))
        add(f"Wc1{l}", _bd(inp["Wc1"][l]))
        add(f"Wc2{l}", _bd_rep(inp["Wc2"][l][:, 0]))
        add(f"Wv1{l}", _bd(inp["Wv1"][l]))
        add(f"Wv2{l}", _bd_rep(inp["Wv2"][l][:, 0]))
        Wn1 = inp["Wn1"][l]          # [128, 64]
        add(f"Wn1t{l}", _bd(Wn1[0:64]))
        add(f"Wn1b{l}", _bd(Wn1[64:128]))
        add(f"Wn1d{l}", _bd(-Wn1[64:128]))
        add(f"Wn2{l}", _bd(inp["Wn2"][l]))

    wpack = np.concatenate(tiles, axis=1)
    widx = {n: i for i, n in enumerate(names)}

    bias_cols = []
    bnames = []
    for l in range(L):
        for nm in ("be1", "be2", "bc1", "bv1", "bn1", "bn2"):
            bias_cols.append(np.tile(inp[nm][l].reshape(-1), 2))
            bnames.append(f"{nm}{l}")
        for nm in ("bv2", "bc2"):
            bias_cols.append(np.full(128, float(inp[nm][l].reshape(-1)[0]), np.float32))
            bnames.append(f"{nm}{l}")
    bias_cols.append(np.tile(inp["emb_b"], 2))
    bnames.append("embb")
    biaspack = np.stack(bias_cols, axis=1).astype(np.float32)  # [128, NB]
    bidx = {n: i for i, n in enumerate(bnames)}
    return wpack, widx, biaspack, bidx


def _arrange_inputs(obs_slice):
    """Per-core obs slice [4096, 20] -> invT [128, 2048], locvel [128, 128]."""
    obs3 = obs_slice.reshape(NGB, 128, 20)          # [gb, (gm,i), col]
    invT = np.zeros((128, NNODE), np.float32)
    # half0: gb 0..15 ; half1: gb 16..31 ; n' = gb_l*128 + p
    inv_half0 = obs3[0:NGBL, :, 0:INV]              # [16, 128, 16]
    inv_half1 = obs3[NGBL:NGB, :, 0:INV]
    invT[0:INV, :] = np.transpose(inv_half0, (2, 0, 1)).reshape(INV, NNODE)
    invT[64:64 + INV, :] = np.transpose(inv_half1, (2, 0, 1)).reshape(INV, NNODE)
    locvel = np.ascontiguousarray(
        np.transpose(obs3[:, :, INV:INV + 4], (1, 0, 2)).reshape(128, NGB * 4)
    ).astype(np.float32)
    return invT, locvel


def _unarrange_output(outP):
    """outP [128, 64] -> [4096, 2] (n = gb*128 + p)."""
    return np.ascontiguousarray(
        outP.reshape(128, NGB, 2).transpose(1, 0, 2).reshape(NODES_CORE, 2)
    )


# ----------------------------------------------------------------------------
# Device kernel builder
# ----------------------------------------------------------------------------

def build(scale0, scale1, mean0, mean1):
    import concourse.bacc as bacc
    import concourse.tile as tile
    import concourse.mybir as mybir
    from contextlib import ExitStack

    F32 = mybir.dt.float32
    F32R = mybir.dt.float32r
    AT = mybir.AluOpType
    ACTF = mybir.ActivationFunctionType

    def r32(ap):
        return ap.bitcast(F32R)

    nc = bacc.Bacc("TRN2", target_bir_lowering=False, debug=False)

    invT_d = nc.dram_tensor("invT", [128, NNODE], F32R, kind="ExternalInput")
    locvel_d = nc.dram_tensor("locvel", [128, NGB * 4], F32, kind="ExternalInput")
    NW = 3 + 12 * L
    wpack_d = nc.dram_tensor("wpack", [128, NW * 128], F32R, kind="ExternalInput")
    NBIAS = 8 * L + 1
    bias_d = nc.dram_tensor("biaspack", [128, NBIAS], F32, kind="ExternalInput")
    out_d = nc.dram_tensor("out", [128, NGB * 2], F32, kind="ExternalOutput")

    # weight tile indices (must match _pack_weights order)
    widx = {}
    _wi = 0
    for nm in ("ident", "delta", "emb"):
        widx[nm] = _wi
        _wi += 1
    for l in range(L):
        for nm in ("Wi", "Wj", "Wsc", "We2", "Wc1", "Wc2", "Wv1", "Wv2",
                   "Wn1t", "Wn1b", "Wn1d", "Wn2"):
            widx[f"{nm}{l}"] = _wi
            _wi += 1
    bidx = {}
    _bi = 0
    for l in range(L):
        for nm in ("be1", "be2", "bc1", "bv1", "bn1", "bn2", "bv2", "bc2"):
            bidx[f"{nm}{l}"] = _bi
            _bi += 1
    bidx["embb"] = _bi

    with tile.TileContext(nc) as tc, ExitStack() as ctx:
        st = ctx.enter_context(tc.tile_pool(name="static", bufs=1))
        eA = ctx.enter_context(tc.tile_pool(name="eA", bufs=2))
        eM = ctx.enter_context(tc.tile_pool(name="eM", bufs=2))
        eS = ctx.enter_context(tc.tile_pool(name="eS", bufs=2))
        eR = ctx.enter_context(tc.tile_pool(name="eR", bufs=2))
        mx = ctx.enter_context(tc.tile_pool(name="mx", bufs=1))
        psA = ctx.enter_context(tc.tile_pool(name="psA", bufs=2, space="PSUM"))
        psB = ctx.enter_context(tc.tile_pool(name="psB", bufs=2, space="PSUM"))
        psC = psB
        psD = psB

        # ---- static loads ----
        wsb = st.tile([128, NW * 128], F32R)
        nc.sync.dma_start(wsb[:], wpack_d.ap())
        bsb = st.tile([128, NBIAS], F32)
        nc.sync.dma_start(bsb[:], bias_d.ap())
        invT = st.tile([128, NNODE], F32R)
        nc.sync.dma_start(invT[:], invT_d.ap())
        locvel = st.tile([128, NGB * 4], F32)
        nc.sync.dma_start(locvel[:], locvel_d.ap())

        def W(name):
            return wsb[:, widx[name] * 128:(widx[name] + 1) * 128]

        def Bia(name):
            return bsb[:, bidx[name]:bidx[name] + 1]

        ident = W("ident").bitcast(F32)
        delta4 = W("delta").bitcast(F32)[0:4, :]

        # ---- persistent state ----
        hA = st.tile([128, NNODE], F32R)
        hB = st.tile([128, NNODE], F32R)
        magg = st.tile([128, NNODE], F32R)
        mdiag = st.tile([128, NNODE], F32R)
        smat = st.tile([128, 1024], F32)
        rad = st.tile([128, 1024], F32R)
        ea = st.tile([128, 1024], F32R)
        dx = st.tile([128, 1024], F32)
        dy = st.tile([128, 1024], F32)
        locx = st.tile([128, NGB], F32)
        locy = st.tile([128, NGB], F32)
        velx = st.tile([128, NGB], F32)
        vely = st.tile([128, NGB], F32)
        phiP = st.tile([128, NGB], F32)
        hv1 = st.tile([128, NNODE], F32R)
        phirep = st.tile([128, NNODE], F32)
        lxT = st.tile([32, 128], F32)
        lyT = st.tile([32, 128], F32)
        T4x = st.tile([4, 1024], F32)
        T4y = st.tile([4, 1024], F32)
        outP = st.tile([128, NGB * 2], F32)

        lv = locvel[:].rearrange("p (gb c) -> p gb c", c=4)
        nc.vector.tensor_copy(locx[:], lv[:, :, 0])
        nc.vector.tensor_copy(locy[:], lv[:, :, 1])
        nc.vector.tensor_copy(velx[:], lv[:, :, 2])
        nc.vector.tensor_copy(vely[:], lv[:, :, 3])

        def heat(lhsT_ap, rhs_ap, n=14):
            hp = psB.tile([128, 512], F32, tag="stage")
            for _ in range(n):
                nc.tensor.matmul(hp[:], lhsT_ap, rhs_ap, start=True, stop=True)

        # ---- embedding: h0 = inv @ emb_W + emb_b ----
        heat(r32(W("emb")), r32(invT[:, 0:512]))
        for u in range(NNODE // 1024):
            pse = psA.tile([128, 1024], F32, tag="psA")
            for k in range(2):
                nc.tensor.matmul(pse[:, k * 512:(k + 1) * 512], r32(W("emb")),
                                 r32(invT[:, u * 1024 + k * 512:u * 1024 + (k + 1) * 512]),
                                 start=True, stop=True)
            nc.vector.tensor_scalar_add(hA[:, u * 1024:(u + 1) * 1024], pse[:], Bia("embb"))

        def radial_part(first):
            """Compute lxT/lyT, T4s, dx, dy, rad from current locx/locy."""
            # transposes (PE) -> SBUF
            for (lP, lT) in ((locx, lxT), (locy, lyT)):
                pst = psD.tile([32, 128], F32, tag="stage")
                nc.tensor.transpose(pst[:], lP[:], ident)
                nc.vector.tensor_copy(lT[:], pst[:])
            # T4 builds: T4[gm', (gb, j)] = lT[gb, gm'*32 + j]
            for (lT, T4) in ((lxT, T4x), (lyT, T4y)):
                for gm in range(4):
                    nc.sync.dma_start(
                        T4[gm:gm + 1, :].rearrange("p (gb j) -> p gb j", j=32),
                        lT[:, gm * 32:(gm + 1) * 32])
            # selector matmuls + dx/dy
            for (T4, lP, dT) in ((T4x, locx, dx), (T4y, locy, dy)):
                pss = psA.tile([128, 1024], F32, tag="psA")
                for k in range(2):
                    nc.tensor.matmul(pss[:, k * 512:(k + 1) * 512], delta4,
                                     T4[:, k * 512:(k + 1) * 512],
                                     start=True, stop=True)
                bc = lP[:].unsqueeze(2).broadcast_to([128, NGB, 32])
                nc.vector.tensor_tensor(
                    dT[:].rearrange("p (gb j) -> p gb j", j=32), bc,
                    pss[:].rearrange("p (gb j) -> p gb j", j=32), op=AT.subtract)
            # rad = dx*dx + dy*dy
            t2 = mx.tile([128, 1024], F32, tag="mx_t2")
            nc.vector.tensor_tensor(rad[:], dx[:], dx[:], op=AT.mult)
            nc.vector.tensor_tensor(t2[:], dy[:], dy[:], op=AT.mult)
            nc.vector.tensor_tensor(rad[:], rad[:], t2[:], op=AT.add)
            if first:
                nc.vector.tensor_copy(ea[:], rad[:])

        radial_part(first=True)

        for l in range(L):
            h = hA if l % 2 == 0 else hB
            h_next = hB if l % 2 == 0 else hA

            # ---- node phase: phi = silu(h@Wv1+bv1)@Wv2 + bv2 -> phiP ----
            for u in range(NNODE // 1024):
                sl = slice(u * 1024, (u + 1) * 1024)
                psv = psC.tile([128, 1024], F32, tag="stage")
                for k in range(2):
                    ksl = slice(u * 1024 + k * 512, u * 1024 + (k + 1) * 512)
                    nc.tensor.matmul(psv[:, k * 512:(k + 1) * 512],
                                     r32(W(f"Wv1{l}")), r32(h[:, ksl]),
                                     start=True, stop=True)
                nc.scalar.activation(hv1[:, sl], psv[:], ACTF.Silu, bias=Bia(f"bv1{l}"))
                psv2 = psC.tile([128, 1024], F32, tag="stage")
                for k in range(2):
                    nc.tensor.matmul(psv2[:, k * 512:(k + 1) * 512],
                                     r32(W(f"Wv2{l}")),
                                     r32(hv1[:, u * 1024 + k * 512:u * 1024 + (k + 1) * 512]),
                                     start=True, stop=True)
                nc.vector.tensor_scalar_add(phirep[:, sl], psv2[:], Bia(f"bv2{l}"))
            for c in range(NGBL):
                pst = psD.tile([128, 128], F32, tag="stage")
                nc.tensor.transpose(pst[:], phirep[:, c * 128:(c + 1) * 128], ident)
                nc.vector.tensor_copy(phiP[:, c:c + 1], pst[:, 0:1])
                nc.vector.tensor_copy(phiP[:, c + NGBL:c + NGBL + 1], pst[:, 64:65])

            # ---- edge phase: 32 chunks of (gb_l, gmp) ----
            for gb_l in range(NGBL):
                for gmp in range(2):
                    n0 = gb_l * 128 + gmp * 64
                    p0 = gmp * 64
                    rsc = eR.tile([4, 2048], F32R, tag="rsc")
                    for (row, src) in ((0, rad), (1, ea)):
                        nc.sync.dma_start(
                            rsc[row:row + 1, :].rearrange(
                                "p (a b c) -> p a b c", a=2, b=32, c=32),
                            src[p0:p0 + 64, gb_l * 32:(gb_l + 1) * 32])
                        nc.sync.dma_start(
                            rsc[row + 2:row + 3, :].rearrange(
                                "p (a b c) -> p a b c", a=2, b=32, c=32),
                            src[p0:p0 + 64, (gb_l + 16) * 32:(gb_l + 17) * 32])
                    if gb_l == 0 and gmp == 0:
                        heat(r32(W(f"Wsc{l}")[0:4, :]), r32(rsc[:, 0:512]))
                    for u in range(2):
                        nb = n0 + u * 32
                        gmg = gmp * 2 + u  # gm of this unit's graph
                        ps1 = psA.tile([128, 1024], F32, tag="psA")
                        for k in range(2):
                            ksl = slice(k * 512, (k + 1) * 512)
                            hi = h[:, nb + k * 16:nb + (k + 1) * 16]
                            hi_bc = hi.unsqueeze(2).broadcast_to([128, 16, 32])
                            hj = h[:, nb:nb + 32]
                            hj_bc = hj.unsqueeze(1).broadcast_to([128, 16, 32])
                            nc.tensor.matmul(ps1[:, ksl], r32(W(f"Wi{l}")),
                                             r32(hi_bc), start=True, stop=False)
                            nc.tensor.matmul(ps1[:, ksl], r32(W(f"Wj{l}")),
                                             r32(hj_bc), start=False, stop=False)
                            nc.tensor.matmul(ps1[:, ksl], r32(W(f"Wsc{l}")[0:4, :]),
                                             r32(rsc[:, u * 1024 + k * 512:u * 1024 + (k + 1) * 512]),
                                             start=False, stop=True)
                        m1s = eA.tile([128, 1024], F32R, tag="m1s")
                        nc.scalar.activation(m1s[:], ps1[:], ACTF.Silu, bias=Bia(f"be1{l}"))
                        ps2 = psB.tile([128, 1024], F32, tag="stage")
                        for k in range(2):
                            ksl = slice(k * 512, (k + 1) * 512)
                            nc.tensor.matmul(ps2[:, ksl], r32(W(f"We2{l}")),
                                             r32(m1s[:, ksl]), start=True, stop=True)
                        m_u = eM.tile([128, 1024], F32R, tag="m_u")
                        nc.scalar.activation(m_u[:], ps2[:], ACTF.Silu, bias=Bia(f"be2{l}"))
                        with nc.allow_low_precision(reason="fp32r magg"):
                            nc.vector.tensor_reduce(
                                magg[:, nb:nb + 32],
                                m_u[:].rearrange("p (i j) -> p i j", j=32),
                                axis=mybir.AxisListType.X, op=AT.add)
                        nc.vector.tensor_copy(mdiag[:, nb:nb + 32], m_u[:, 0:1024:33])
                        ps3 = psC.tile([128, 1024], F32, tag="stage")
                        for k in range(2):
                            ksl = slice(k * 512, (k + 1) * 512)
                            nc.tensor.matmul(ps3[:, ksl], r32(W(f"Wc1{l}")),
                                             r32(m_u[:, ksl]), start=True, stop=True)
                        c1 = eA.tile([128, 1024], F32R, tag="c1")
                        nc.scalar.activation(c1[:], ps3[:], ACTF.Silu, bias=Bia(f"bc1{l}"))
                        ps4 = psD.tile([128, 1024], F32, tag="stage")
                        for k in range(2):
                            ksl = slice(k * 512, (k + 1) * 512)
                            nc.tensor.matmul(ps4[:, ksl], r32(W(f"Wc2{l}")),
                                             r32(c1[:, ksl]), start=True, stop=True)
                        ssb = eS.tile([128, 1024], F32, tag="ssb")
                        nc.vector.tensor_scalar_add(ssb[:], ps4[:], Bia(f"bc2{l}"))
                        # s -> matrix layout (per half)
                        pg = gmg * 32
                        nc.sync.dma_start(
                            smat[pg:pg + 32, gb_l * 32:(gb_l + 1) * 32],
                            ssb[0:1, :].rearrange("p (i j) -> p i j", j=32))
                        nc.sync.dma_start(
                            smat[pg:pg + 32, (gb_l + 16) * 32:(gb_l + 17) * 32],
                            ssb[64:65, :].rearrange("p (i j) -> p i j", j=32))

            # ---- h update ----
            for u in range(NNODE // 1024):
                sl = slice(u * 1024, (u + 1) * 1024)
                psh = psB.tile([128, 1024], F32, tag="stage")
                for k in range(2):
                    ksl = slice(u * 1024 + k * 512, u * 1024 + (k + 1) * 512)
                    osl = slice(k * 512, (k + 1) * 512)
                    nc.tensor.matmul(psh[:, osl], r32(W(f"Wn1t{l}")),
                                     r32(h[:, ksl]), start=True, stop=False)
                    nc.tensor.matmul(psh[:, osl], r32(W(f"Wn1b{l}")),
                                     r32(magg[:, ksl]), start=False, stop=False)
                    nc.tensor.matmul(psh[:, osl], r32(W(f"Wn1d{l}")),
                                     r32(mdiag[:, ksl]), start=False, stop=True)
                hn1 = eA.tile([128, 1024], F32R, tag="hn1")
                nc.scalar.activation(hn1[:], psh[:], ACTF.Silu, bias=Bia(f"bn1{l}"))
                psh2 = psB.tile([128, 1024], F32, tag="stage")
                for k in range(2):
                    osl = slice(k * 512, (k + 1) * 512)
                    nc.tensor.matmul(psh2[:, osl], r32(W(f"Wn2{l}")),
                                     r32(hn1[:, osl]), start=True, stop=True)
                nc.vector.scalar_tensor_tensor(
                    h_next[:, sl], psh2[:], Bia(f"bn2{l}"), h[:, sl],
                    op0=AT.add, op1=AT.add)

            # ---- matrix phase: t, u, agg, vel/loc update; then radial(l+1) ----
            sq = mx.tile([128, 1024], F32, tag="mx_sq")
            nc.scalar.activation(sq[:], rad[:], ACTF.Sqrt)
            nc.vector.tensor_scalar_add(sq[:], sq[:], 1.0)
            tm = mx.tile([128, 1024], F32, tag="mx_tm")
            nc.vector.reciprocal(tm[:], sq[:])
            um = mx.tile([128, 1024], F32, tag="mx_um")
            nc.vector.tensor_tensor(um[:], tm[:], smat[:], op=AT.mult)
            for (dT, agg_out) in ((dx, "ax"), (dy, "ay")):
                w_ = mx.tile([128, 1024], F32, tag="mx_w")
                nc.vector.tensor_tensor(w_[:], um[:], dT[:], op=AT.mult)
                ag = mx.tile([128, NGB], F32, tag="mx_" + agg_out)
                nc.vector.tensor_reduce(
                    ag[:], w_[:].rearrange("p (gb j) -> p gb j", j=32),
                    axis=mybir.AxisListType.X, op=AT.add)
                vP = velx if agg_out == "ax" else vely
                tmp = mx.tile([128, NGB], F32, tag="mx_tmp")
                nc.vector.tensor_tensor(tmp[:], phiP[:], vP[:], op=AT.mult)
                nc.vector.scalar_tensor_tensor(vP[:], ag[:], 1.0 / DEG, tmp[:],
                                               op0=AT.mult, op1=AT.add)
            nc.vector.tensor_tensor(locx[:], locx[:], velx[:], op=AT.add)
            nc.vector.tensor_tensor(locy[:], locy[:], vely[:], op=AT.add)
            if l < L - 1:
                radial_part(first=False)

        # ---- output: outP interleaved (gb, c) ----
        ov = outP[:].rearrange("p (gb c) -> p gb c", c=2)
        nc.vector.tensor_scalar(ov[:, :, 0], velx[:], scale0, mean0,
                                op0=AT.mult, op1=AT.add)
        nc.vector.tensor_scalar(ov[:, :, 1], vely[:], scale1, mean1,
                                op0=AT.mult, op1=AT.add)
        nc.sync.dma_start(out_d.ap(), outP[:])

    nc.compile()
    return nc


# ----------------------------------------------------------------------------
# Entry point
# ----------------------------------------------------------------------------

def kernel(**inputs):
    import concourse.mybir  # noqa: F401  (ensure env importable)
    from concourse.bass_utils import run_bass_kernel_spmd

    inp = {k: np.asarray(v) for k, v in inputs.items()}
    obs = inp["obs"].astype(np.float32)
    scale = np.asarray(inp["scale"], np.float32)
    mean = np.asarray(inp["mean"], np.float32)

    key = (float(scale[0]), float(scale[1]), float(mean[0]), float(mean[1]))
    if key not in _BUILD_CACHE:
        _BUILD_CACHE[key] = build(*key)
    nc = _BUILD_CACHE[key]

    wpack, _, biaspack, _ = _pack_weights(inp)
    in_maps = []
    for c in range(NCORES):
        invT, locvel = _arrange_inputs(obs[c * NODES_CORE:(c + 1) * NODES_CORE])
        in_maps.append({"invT": invT, "locvel": locvel,
                        "wpack": wpack, "biaspack": biaspack})
    res = run_bass_kernel_spmd(nc, in_maps, list(range(NCORES)))
    outs = [_unarrange_output(res.results[c]["out"]) for c in range(NCORES)]
    return np.concatenate(outs, axis=0)



# revision 25
# speedup vs baseline: 2.7446x; 2.7446x over previous
"""Trainium2 Bass kernel for nn_DeterministicEgnnPolicy (EGNN message passing).

Strategy: shard the 1024 independent 32-node graphs across 8 NeuronCores
(128 graphs/core).  On each core:

- Node features h live feature-major in fp16: [128 = 2 halves x 64 feats,
  cols n = (k in 64, i in 32)].  Graph g = half*64 + k.  A "unit" k pairs
  graphs (k, 64+k); block-diagonal fp16 weights process both at once with a
  full 128-partition contraction (fp16 moving operands stream 1024 cols at
  1 cycle/row and keep 8x finer mantissa than bf16).
- Per-edge/per-node scalars live graph-major: [128 = g, cols (i, j)] resp.
  [128 = g, i].  All geometry (radial, trans, agg, vel, loc) is computed
  with broadcast-AP elementwise ops -- no transposes, no selector matmuls.
- Layout crossings are cheap 1-2 descriptor DMAs: rsc (radial rows for the
  edge MLP) and smat (per-edge scalar s), plus one 128-descriptor DMA per
  layer for phi.
- The edge MLP runs as a software-pipelined loop over 64 units of 1024
  edge-columns, stages shifted so PE / Scalar(ACT) / DVE / GpSimd / Sync
  all stream without per-unit round trips.  PSUM: 4 stage pools x 1 buf x
  2 banks = 8 banks exactly.  GpSimd owns magg/mdiag and half the matrix
  phase.
"""

import numpy as np

FP16NP = np.float16

N_AGENTS = 32
BATCH = 1024
H = 64
L = 4
INV = 16
DEG = float(N_AGENTS - 1)
NCORES = 8
G_CORE = BATCH // NCORES          # 128 graphs per core
NU = G_CORE // 2                  # 64 units (graph pairs) per core
NNODE = NU * 32                   # 2048 node cols (per half)
NODES_CORE = G_CORE * N_AGENTS    # 4096
NE = 1024                         # edge cols per graph (32*32 dense)

_BUILD_CACHE = {}


# ----------------------------------------------------------------------------
# Host-side packing (pure layout permutation / weight arrangement)
# ----------------------------------------------------------------------------

def _bd(w):
    """64x64 block-diagonal lhsT [128,128] from w [k,64]."""
    k = w.shape[0]
    out = np.zeros((128, 128), np.float32)
    out[0:k, 0:64] = w
    out[64:64 + k, 64:128] = w
    return out


def _bd_rep(wcol):
    """Replicating lhsT: out[64h+f, 64h+f'] = wcol[f] for all f'."""
    out = np.zeros((128, 128), np.float32)
    col = wcol.reshape(64, 1)
    out[0:64, 0:64] = np.repeat(col, 64, axis=1)
    out[64:128, 64:128] = np.repeat(col, 64, axis=1)
    return out


WNAMES = ("Wi", "Wj", "Wsc", "We2", "Wc1", "Wc2", "Wv1", "Wv2",
          "Wn1t", "Wn1b", "Wn1d", "Wn2")
BNAMES = ("be1", "be2", "bc1", "bv1", "bn1", "bn2", "bv2", "bc2")


def _pack_weights(inp):
    """Build wpack [128, NW*128] fp16 and biaspack [128, NBIAS] f32."""
    tiles = []

    def add(arr):
        t = np.zeros((128, 128), np.float32)
        t[:arr.shape[0], :arr.shape[1]] = arr
        tiles.append(t)

    emb = np.zeros((128, 128), np.float32)
    emb[0:INV, 0:64] = inp["emb_W"]
    emb[64:64 + INV, 64:128] = inp["emb_W"]
    add(emb)

    for l in range(L):
        We1 = inp["We1"][l]          # [130, 64]
        add(_bd(We1[0:64]))          # Wi
        add(_bd(We1[64:128]))        # Wj
        wsc = np.zeros((4, 128), np.float32)
        wsc[0, 0:64] = We1[128]      # radial, half0 (graph k)
        wsc[1, 0:64] = We1[129]      # edge_attr, half0
        wsc[2, 64:128] = We1[128]    # radial, half1 (graph 64+k)
        wsc[3, 64:128] = We1[129]
        add(wsc)
        add(_bd(inp["We2"][l]))
        add(_bd(inp["Wc1"][l]))
        add(_bd_rep(inp["Wc2"][l][:, 0]))
        add(_bd(inp["Wv1"][l]))
        add(_bd_rep(inp["Wv2"][l][:, 0]))
        Wn1 = inp["Wn1"][l]          # [128, 64]
        add(_bd(Wn1[0:64]))          # Wn1t
        add(_bd(Wn1[64:128]))        # Wn1b
        add(_bd(-Wn1[64:128]))       # Wn1d
        add(_bd(inp["Wn2"][l]))

    wpack = np.concatenate(tiles, axis=1).astype(FP16NP)

    bias_cols = []
    for l in range(L):
        for nm in ("be1", "be2", "bc1", "bv1", "bn1", "bn2"):
            bias_cols.append(np.tile(np.asarray(inp[nm][l]).reshape(-1), 2))
        for nm in ("bv2", "bc2"):
            bias_cols.append(
                np.full(128, float(np.asarray(inp[nm][l]).reshape(-1)[0]),
                        np.float32))
    bias_cols.append(np.tile(inp["emb_b"], 2))
    biaspack = np.stack(bias_cols, axis=1).astype(np.float32)
    return wpack, None, biaspack, None


def _arrange_inputs(obs_slice):
    """Per-core obs [4096, 20] -> invT fp16 [128,2048], locvelG f32 [128,128]."""
    obs3 = np.asarray(obs_slice, np.float32).reshape(G_CORE, N_AGENTS, INV + 4)
    invT = np.zeros((128, NNODE), np.float32)
    inv = obs3[:, :, 0:INV]                      # [g, i, f]
    invT[0:INV, :] = inv[0:64].transpose(2, 0, 1).reshape(INV, NNODE)
    invT[64:64 + INV, :] = inv[64:128].transpose(2, 0, 1).reshape(INV, NNODE)
    locvelG = np.ascontiguousarray(
        obs3[:, :, INV:INV + 4].reshape(G_CORE, 4 * N_AGENTS))
    return invT.astype(FP16NP), locvelG


def _unarrange_output(outP):
    """outP [128, 64] (g, (i,c)) -> [4096, 2]."""
    return np.asarray(outP, np.float32).reshape(NODES_CORE, 2)


# ----------------------------------------------------------------------------
# Device kernel builder
# ----------------------------------------------------------------------------

def build(scale0, scale1, mean0, mean1):
    import concourse.bacc as bacc
    import concourse.tile as tile
    import concourse.mybir as mybir
    from contextlib import ExitStack

    F32 = mybir.dt.float32
    FP16 = mybir.dt.float16
    AT = mybir.AluOpType
    ACTF = mybir.ActivationFunctionType
    AX = mybir.AxisListType.X

    nc = bacc.Bacc("TRN2", target_bir_lowering=False, debug=False)

    invT_d = nc.dram_tensor("invT", [128, NNODE], FP16, kind="ExternalInput")
    locvel_d = nc.dram_tensor("locvel", [128, 4 * N_AGENTS], F32,
                              kind="ExternalInput")
    NW = 1 + 12 * L
    wpack_d = nc.dram_tensor("wpack", [128, NW * 128], FP16,
                             kind="ExternalInput")
    NBIAS = 8 * L + 1
    bias_d = nc.dram_tensor("biaspack", [128, NBIAS], F32,
                            kind="ExternalInput")
    out_d = nc.dram_tensor("out", [128, 2 * N_AGENTS], F32,
                           kind="ExternalOutput")

    widx = {"emb": 0}
    _wi = 1
    for l in range(L):
        for nm in WNAMES:
            widx[f"{nm}{l}"] = _wi
            _wi += 1
    bidx = {}
    _bi = 0
    for l in range(L):
        for nm in BNAMES:
            bidx[f"{nm}{l}"] = _bi
            _bi += 1
    bidx["embb"] = _bi

    with tile.TileContext(nc) as tc, ExitStack() as ctx:
        st = ctx.enter_context(tc.tile_pool(name="static", bufs=1))
        p1 = ctx.enter_context(tc.tile_pool(name="p1", bufs=1, space="PSUM"))
        p2 = ctx.enter_context(tc.tile_pool(name="p2", bufs=1, space="PSUM"))
        p3 = ctx.enter_context(tc.tile_pool(name="p3", bufs=1, space="PSUM"))
        p4 = ctx.enter_context(tc.tile_pool(name="p4", bufs=1, space="PSUM"))
        em1 = ctx.enter_context(tc.tile_pool(name="em1", bufs=2))
        em2 = ctx.enter_context(tc.tile_pool(name="em2", bufs=3))
        ec1 = ctx.enter_context(tc.tile_pool(name="ec1", bufs=2))
        esb = ctx.enter_context(tc.tile_pool(name="esb", bufs=2))
        ersc = ctx.enter_context(tc.tile_pool(name="ersc", bufs=4))

        # ---- static loads ----
        wsb = st.tile([128, NW * 128], FP16)
        nc.sync.dma_start(wsb[:], wpack_d.ap())
        bsb = st.tile([128, NBIAS], F32)
        nc.sync.dma_start(bsb[:], bias_d.ap())
        invT = st.tile([128, NNODE], FP16)
        nc.sync.dma_start(invT[:], invT_d.ap())
        locvel = st.tile([128, 4 * N_AGENTS], F32)
        nc.sync.dma_start(locvel[:], locvel_d.ap())

        def W(name):
            return wsb[:, widx[name] * 128:(widx[name] + 1) * 128]

        def Bia(name):
            return bsb[:, bidx[name]:bidx[name] + 1]

        # ---- persistent state ----
        hA = st.tile([128, NNODE], FP16)
        hB = st.tile([128, NNODE], FP16)
        magg = st.tile([128, NNODE], FP16)
        mdiag = st.tile([128, NNODE], FP16)
        hv1 = st.tile([128, NNODE], FP16)
        phirep = st.tile([128, NNODE], F32)
        phiG = st.tile([128, N_AGENTS], F32)
        locx = st.tile([128, N_AGENTS], F32)
        locy = st.tile([128, N_AGENTS], F32)
        velx = st.tile([128, N_AGENTS], F32)
        vely = st.tile([128, N_AGENTS], F32)
        tvx = st.tile([128, N_AGENTS], F32)
        tvy = st.tile([128, N_AGENTS], F32)
        aggx = st.tile([128, N_AGENTS], FP16)
        aggy = st.tile([128, N_AGENTS], FP16)
        rad = st.tile([128, NE], FP16)
        dx = st.tile([128, NE], FP16)
        dy = st.tile([128, NE], FP16)
        sq = st.tile([128, NE], FP16)
        tm = st.tile([128, NE], FP16)
        um = st.tile([128, NE], FP16)
        wt = st.tile([128, NE], FP16)
        wt2 = st.tile([128, NE], FP16)
        smat = st.tile([128, NE], FP16)
        ea = st.tile([128, NE], FP16)
        outP = st.tile([128, 2 * N_AGENTS], F32)

        def heat(n=14):
            hp = p1.tile([128, NE], F32, tag="ps1")
            for _ in range(n):
                nc.tensor.matmul(hp[:, 0:512], W("emb"), invT[:, 0:512],
                                 start=True, stop=True)

        heat()

        # ---- embedding: h0 = inv @ emb_W + emb_b ----
        for c in range(2):
            sl = slice(c * NE, (c + 1) * NE)
            pse = p2.tile([128, NE], F32, tag="ps2")
            for ih in range(2):
                osl = slice(ih * 512, (ih + 1) * 512)
                isl = slice(c * NE + ih * 512, c * NE + (ih + 1) * 512)
                nc.tensor.matmul(pse[:, osl], W("emb"), invT[:, isl],
                                 start=True, stop=True)
            with nc.allow_low_precision(reason="fp16 h"):
                nc.vector.tensor_scalar_add(hA[:, sl], pse[:], Bia("embb"))

        # ---- initial geometry (graph-major) ----
        lv = locvel[:].rearrange("p (i c) -> p i c", c=4)
        nc.vector.tensor_copy(locx[:], lv[:, :, 0])
        nc.vector.tensor_copy(locy[:], lv[:, :, 1])
        nc.vector.tensor_copy(velx[:], lv[:, :, 2])
        nc.vector.tensor_copy(vely[:], lv[:, :, 3])

        def rad_chain(first):
            """dx, dy, rad from locx/locy (fp16 out; DVE + GpSimd)."""
            with nc.allow_low_precision(reason="fp16 geometry"):
                for (lP, dT, eng) in ((locx, dx, nc.vector),
                                      (locy, dy, nc.vector)):
                    bi = lP[:].unsqueeze(2).broadcast_to(
                        [128, N_AGENTS, N_AGENTS])
                    bj = lP[:].unsqueeze(1).broadcast_to(
                        [128, N_AGENTS, N_AGENTS])
                    eng.tensor_tensor(
                        dT[:].rearrange("p (i j) -> p i j", j=N_AGENTS),
                        bi, bj, op=AT.subtract)
                nc.vector.tensor_tensor(wt2[:], dy[:], dy[:], op=AT.mult)
                nc.vector.tensor_tensor(rad[:], dx[:], dx[:], op=AT.mult)
                nc.vector.tensor_tensor(rad[:], rad[:], wt2[:], op=AT.add)
                if first:
                    nc.vector.tensor_copy(ea[:], rad[:])

        rad_chain(first=True)

        def prefetch_rsc(k):
            rsc = ersc.tile([4, NE], FP16, tag="rsc")
            nc.sync.dma_start(rsc[0:3:2, :], rad[k:k + 65:64, :])
            nc.sync.dma_start(rsc[1:4:2, :], ea[k:k + 65:64, :])
            return rsc

        for l in range(L):
            h = hA if l % 2 == 0 else hB
            h_next = hB if l % 2 == 0 else hA

            # ---- node phase: phi = silu(h@Wv1+bv1)@Wv2 + bv2 ----
            psv_q = [None] * 2
            for c in range(3):
                if c >= 1:
                    cc = c - 1
                    sl = slice(cc * NE, (cc + 1) * NE)
                    with nc.allow_low_precision(reason="fp16 hv1"):
                        nc.scalar.activation(hv1[:, sl], psv_q[cc][:],
                                             ACTF.Silu, bias=Bia(f"bv1{l}"))
                    psv2 = p2.tile([128, NE], F32, tag="ps2")
                    for ih in range(2):
                        osl = slice(ih * 512, (ih + 1) * 512)
                        isl = slice(cc * NE + ih * 512, cc * NE + (ih + 1) * 512)
                        nc.tensor.matmul(psv2[:, osl], W(f"Wv2{l}"),
                                         hv1[:, isl], start=True, stop=True)
                    nc.vector.tensor_scalar_add(phirep[:, sl], psv2[:],
                                                Bia(f"bv2{l}"))
                    psv_q[cc] = None
                if c < 2:
                    sl = slice(c * NE, (c + 1) * NE)
                    psv = p1.tile([128, NE], F32, tag="ps1")
                    for ih in range(2):
                        osl = slice(ih * 512, (ih + 1) * 512)
                        isl = slice(c * NE + ih * 512, c * NE + (ih + 1) * 512)
                        nc.tensor.matmul(psv[:, osl], W(f"Wv1{l}"),
                                         h[:, isl], start=True, stop=True)
                    psv_q[c] = psv
            # phi -> graph-major [g, i] (128 x 128B descriptors)
            nc.sync.dma_start(phiG[:, :], phirep[0:65:64, :])

            # ---- tm = 1/(1+sqrt(rad)) (overlaps edge phase on DVE) ----
            nc.scalar.activation(sq[:], rad[:], ACTF.Sqrt)
            with nc.allow_low_precision(reason="fp16 tm"):
                nc.vector.tensor_scalar_add(sq[:], sq[:], 1.0)
                nc.vector.reciprocal(tm[:], sq[:])

            # ---- edge phase: software-pipelined over 64 units ----
            rsc_q = [None] * NU
            ps1_q = [None] * NU
            ps2_q = [None] * NU
            ps3_q = [None] * NU
            ps4_q = [None] * NU
            m1_q = [None] * NU
            m_q = [None] * NU

            rsc_q[0] = prefetch_rsc(0)
            rsc_q[1] = prefetch_rsc(1)

            # Emission order per iteration: each PSUM pool's consumer comes
            # before that pool's next producer (bufs=1 slot recycling).
            for t in range(NU + 4):
                if 4 <= t:
                    # S8: evacuate ps4(t-4) -> SBUF; smat rows {k, 64+k}
                    k = t - 4
                    ssb = esb.tile([128, NE], FP16, tag="ssb")
                    with nc.allow_low_precision(reason="fp16 s"):
                        nc.vector.tensor_copy(ssb[:], ps4_q[k][:])
                    nc.sync.dma_start(smat[k:k + 65:64, :], ssb[0:65:64, :])
                    ps4_q[k] = None
                    rsc_q[k] = None
                if 3 <= t and t - 3 < NU:
                    # S6: ACT c1(t-3) frees p3; S7: ps4(t-3) fills p4
                    k = t - 3
                    c1 = ec1.tile([128, NE], FP16, tag="c1")
                    nc.scalar.activation(c1[:], ps3_q[k][:],
                                         ACTF.Silu, bias=Bia(f"bc1{l}"))
                    ps = p4.tile([128, NE], F32, tag="ps4")
                    for ih in range(2):
                        osl = slice(ih * 512, (ih + 1) * 512)
                        nc.tensor.matmul(ps[:, osl], W(f"Wc2{l}"),
                                         c1[:, osl], start=True, stop=True)
                    ps4_q[k] = ps
                    ps3_q[k] = None
                    m_q[k] = None
                if 2 <= t and t - 2 < NU:
                    # S4: ACT m(t-2) frees p2; S5: ps3(t-2) fills p3;
                    # magg/mdiag on GpSimd
                    k = t - 2
                    nb = k * 32
                    m_u = em2.tile([128, NE], FP16, tag="m")
                    nc.scalar.activation(m_u[:], ps2_q[k][:],
                                         ACTF.Silu, bias=Bia(f"be2{l}"))
                    ps = p3.tile([128, NE], F32, tag="ps3")
                    for ih in range(2):
                        osl = slice(ih * 512, (ih + 1) * 512)
                        nc.tensor.matmul(ps[:, osl], W(f"Wc1{l}"),
                                         m_u[:, osl], start=True, stop=True)
                    with nc.allow_low_precision(reason="fp16 magg"):
                        nc.vector.tensor_reduce(
                            magg[:, nb:nb + 32],
                            m_u[:].rearrange("p (i j) -> p i j", j=32),
                            axis=AX, op=AT.add)
                        nc.vector.tensor_copy(mdiag[:, nb:nb + 32],
                                              m_u[:, 0:NE:33])
                    m_q[k] = m_u
                    ps3_q[k] = ps
                    ps2_q[k] = None
                if 1 <= t <= NU:
                    # S2: ACT m1(t-1) frees p1; S3: ps2(t-1) fills p2
                    k = t - 1
                    m1 = em1.tile([128, NE], FP16, tag="m1")
                    nc.scalar.activation(m1[:], ps1_q[k][:],
                                         ACTF.Silu, bias=Bia(f"be1{l}"))
                    ps = p2.tile([128, NE], F32, tag="ps2")
                    for ih in range(2):
                        osl = slice(ih * 512, (ih + 1) * 512)
                        nc.tensor.matmul(ps[:, osl], W(f"We2{l}"),
                                         m1[:, osl], start=True, stop=True)
                    ps2_q[k] = ps
                    ps1_q[k] = None
                    m1_q[k] = None
                if t < NU:
                    # S1: ps1(t) = Wi@h_i + Wj@h_j + Wsc@[r, ea]
                    # (matmul outputs are capped at 512 cols -> half pairs)
                    k = t
                    nb = k * 32
                    ps = p1.tile([128, NE], F32, tag="ps1")
                    hj = h[:, nb:nb + 32]
                    hj_bc = hj.unsqueeze(1).broadcast_to([128, 16, 32])
                    for ih in range(2):
                        osl = slice(ih * 512, (ih + 1) * 512)
                        hi = h[:, nb + ih * 16:nb + ih * 16 + 16]
                        hi_bc = hi.unsqueeze(2).broadcast_to([128, 16, 32])
                        nc.tensor.matmul(ps[:, osl], W(f"Wi{l}"), hi_bc,
                                         start=True, stop=False)
                        nc.tensor.matmul(ps[:, osl], W(f"Wj{l}"), hj_bc,
                                         start=False, stop=False)
                        nc.tensor.matmul(ps[:, osl], W(f"Wsc{l}")[0:4, :],
                                         rsc_q[k][:, osl],
                                         start=False, stop=True)
                    ps1_q[k] = ps
                if t + 2 < NU:
                    rsc_q[t + 2] = prefetch_rsc(t + 2)

            # ---- h update: h' = h + silu([h|magg]@Wn1+bn1)@Wn2 + bn2 ----
            psh_q = [None] * 2
            psh2_q = [None] * 2
            for c in range(4):
                if 2 <= c:
                    cc = c - 2
                    sl = slice(cc * NE, (cc + 1) * NE)
                    with nc.allow_low_precision(reason="fp16 h"):
                        nc.vector.scalar_tensor_tensor(
                            h_next[:, sl], psh2_q[cc][:], Bia(f"bn2{l}"),
                            h[:, sl], op0=AT.add, op1=AT.add)
                    psh2_q[cc] = None
                if 1 <= c <= 2:
                    cc = c - 1
                    sl = slice(cc * NE, (cc + 1) * NE)
                    hn1 = em1.tile([128, NE], FP16, tag="m1")
                    nc.scalar.activation(hn1[:], psh_q[cc][:], ACTF.Silu,
                                         bias=Bia(f"bn1{l}"))
                    psh2 = p4.tile([128, NE], F32, tag="ps4")
                    for ih in range(2):
                        osl = slice(ih * 512, (ih + 1) * 512)
                        nc.tensor.matmul(psh2[:, osl], W(f"Wn2{l}"),
                                         hn1[:, osl], start=True, stop=True)
                    psh2_q[cc] = psh2
                    psh_q[cc] = None
                if c < 2:
                    sl = slice(c * NE, (c + 1) * NE)
                    psh = p3.tile([128, NE], F32, tag="ps3")
                    for ih in range(2):
                        osl = slice(ih * 512, (ih + 1) * 512)
                        isl = slice(c * NE + ih * 512, c * NE + (ih + 1) * 512)
                        nc.tensor.matmul(psh[:, osl], W(f"Wn1t{l}"),
                                         h[:, isl], start=True, stop=False)
                        nc.tensor.matmul(psh[:, osl], W(f"Wn1b{l}"),
                                         magg[:, isl], start=False, stop=False)
                        nc.tensor.matmul(psh[:, osl], W(f"Wn1d{l}"),
                                         mdiag[:, isl], start=False, stop=True)
                    psh_q[c] = psh

            # ---- matrix phase: u, agg, vel/loc update (DVE x, GpSimd y) --
            with nc.allow_low_precision(reason="fp16 matrix"):
                nc.vector.scalar_tensor_tensor(um[:], smat[:], Bia(f"bc2{l}"),
                                               tm[:], op0=AT.add, op1=AT.mult)
                nc.vector.tensor_tensor(wt2[:], um[:], dy[:], op=AT.mult)
                nc.vector.tensor_tensor(wt[:], um[:], dx[:], op=AT.mult)
                for (lP, vP, ag, tv, w_) in (
                        (locx, velx, aggx, tvx, wt),
                        (locy, vely, aggy, tvy, wt2)):
                    nc.vector.tensor_reduce(
                        ag[:], w_[:].rearrange("p (i j) -> p i j", j=32),
                        axis=AX, op=AT.add)
                    nc.vector.tensor_tensor(tv[:], phiG[:], vP[:], op=AT.mult)
                    nc.vector.scalar_tensor_tensor(vP[:], ag[:], 1.0 / DEG,
                                                   tv[:], op0=AT.mult,
                                                   op1=AT.add)
                    nc.vector.tensor_tensor(lP[:], lP[:], vP[:], op=AT.add)
            if l < L - 1:
                rad_chain(first=False)

        # ---- output: outP[g, (i,c)] = scale * vel + mean ----
        ov = outP[:].rearrange("p (i c) -> p i c", c=2)
        nc.vector.tensor_scalar(ov[:, :, 0], velx[:], scale0, mean0,
                                op0=AT.mult, op1=AT.add)
        nc.vector.tensor_scalar(ov[:, :, 1], vely[:], scale1, mean1,
                                op0=AT.mult, op1=AT.add)
        nc.sync.dma_start(out_d.ap(), outP[:])

    nc.compile()
    return nc


# ----------------------------------------------------------------------------
# Entry point
# ----------------------------------------------------------------------------

def kernel(**inputs):
    import concourse.mybir  # noqa: F401  (ensure env importable)
    from concourse.bass_utils import run_bass_kernel_spmd

    inp = {k: np.asarray(v) for k, v in inputs.items()}
    obs = inp["obs"].astype(np.float32)
    scale = np.asarray(inp["scale"], np.float32)
    mean = np.asarray(inp["mean"], np.float32)

    key = (float(scale[0]), float(scale[1]), float(mean[0]), float(mean[1]))
    if key not in _BUILD_CACHE:
        _BUILD_CACHE[key] = build(*key)
    nc = _BUILD_CACHE[key]

    wpack, _, biaspack, _ = _pack_weights(inp)
    in_maps = []
    for c in range(NCORES):
        invT, locvel = _arrange_inputs(obs[c * NODES_CORE:(c + 1) * NODES_CORE])
        in_maps.append({"invT": invT, "locvel": locvel,
                        "wpack": wpack, "biaspack": biaspack})
    res = run_bass_kernel_spmd(nc, in_maps, list(range(NCORES)))
    outs = [_unarrange_output(res.results[c]["out"]) for c in range(NCORES)]
    return np.concatenate(outs, axis=0)


# revision 26
# speedup vs baseline: 2.9906x; 1.0896x over previous
"""Trainium2 Bass kernel for nn_DeterministicEgnnPolicy (EGNN message passing).

Strategy: shard the 1024 independent 32-node graphs across 8 NeuronCores
(128 graphs/core).  On each core:

- Node features h live feature-major in fp16: [128 = 2 halves x 64 feats,
  cols n = (k in 64, i in 32)].  Graph g = half*64 + k.  A "unit" k pairs
  graphs (k, 64+k); block-diagonal fp16 weights process both at once with a
  full 128-partition contraction (fp16 moving operands stream 1024 cols at
  1 cycle/row and keep 8x finer mantissa than bf16).
- Per-edge/per-node scalars live graph-major: [128 = g, cols (i, j)] resp.
  [128 = g, i].  All geometry (radial, trans, agg, vel, loc) is computed
  with broadcast-AP elementwise ops -- no transposes, no selector matmuls.
- Layout crossings are cheap 1-2 descriptor DMAs: rsc (radial rows for the
  edge MLP) and smat (per-edge scalar s), plus one 128-descriptor DMA per
  layer for phi.
- The edge MLP runs as a software-pipelined loop over 64 units of 1024
  edge-columns, stages shifted so PE / Scalar(ACT) / DVE / GpSimd / Sync
  all stream without per-unit round trips.  PSUM: 4 stage pools x 1 buf x
  2 banks = 8 banks exactly.  GpSimd owns magg/mdiag and half the matrix
  phase.
"""

import numpy as np

FP16NP = np.float16

N_AGENTS = 32
BATCH = 1024
H = 64
L = 4
INV = 16
DEG = float(N_AGENTS - 1)
NCORES = 8
G_CORE = BATCH // NCORES          # 128 graphs per core
NU = G_CORE // 2                  # 64 units (graph pairs) per core
NNODE = NU * 32                   # 2048 node cols (per half)
NODES_CORE = G_CORE * N_AGENTS    # 4096
NE = 1024                         # edge cols per graph (32*32 dense)

_BUILD_CACHE = {}


# ----------------------------------------------------------------------------
# Host-side packing (pure layout permutation / weight arrangement)
# ----------------------------------------------------------------------------

def _bd(w):
    """64x64 block-diagonal lhsT [128,128] from w [k,64]."""
    k = w.shape[0]
    out = np.zeros((128, 128), np.float32)
    out[0:k, 0:64] = w
    out[64:64 + k, 64:128] = w
    return out


def _bd_rep(wcol):
    """Replicating lhsT: out[64h+f, 64h+f'] = wcol[f] for all f'."""
    out = np.zeros((128, 128), np.float32)
    col = wcol.reshape(64, 1)
    out[0:64, 0:64] = np.repeat(col, 64, axis=1)
    out[64:128, 64:128] = np.repeat(col, 64, axis=1)
    return out


WNAMES = ("Wi", "Wj", "Wsc", "We2", "Wc1", "Wc2", "Wv1", "Wv2",
          "Wn1t", "Wn1b", "Wn1d", "Wn2")
BNAMES = ("be1", "be2", "bc1", "bv1", "bn1", "bn2", "bv2", "bc2")


def _pack_weights(inp):
    """Build wpack [128, NW*128] fp16 and biaspack [128, NBIAS] f32."""
    tiles = []

    def add(arr):
        t = np.zeros((128, 128), np.float32)
        t[:arr.shape[0], :arr.shape[1]] = arr
        tiles.append(t)

    emb = np.zeros((128, 128), np.float32)
    emb[0:INV, 0:64] = inp["emb_W"]
    emb[64:64 + INV, 64:128] = inp["emb_W"]
    add(emb)

    for l in range(L):
        We1 = inp["We1"][l]          # [130, 64]
        add(_bd(We1[0:64]))          # Wi
        add(_bd(We1[64:128]))        # Wj
        wsc = np.zeros((4, 128), np.float32)
        wsc[0, 0:64] = We1[128]      # radial, half0 (graph k)
        wsc[1, 0:64] = We1[129]      # edge_attr, half0
        wsc[2, 64:128] = We1[128]    # radial, half1 (graph 64+k)
        wsc[3, 64:128] = We1[129]
        add(wsc)
        add(_bd(inp["We2"][l]))
        add(_bd(inp["Wc1"][l]))
        add(_bd_rep(inp["Wc2"][l][:, 0]))
        add(_bd(inp["Wv1"][l]))
        add(_bd_rep(inp["Wv2"][l][:, 0]))
        Wn1 = inp["Wn1"][l]          # [128, 64]
        add(_bd(Wn1[0:64]))          # Wn1t
        add(_bd(Wn1[64:128]))        # Wn1b
        add(_bd(-Wn1[64:128]))       # Wn1d
        add(_bd(inp["Wn2"][l]))

    wpack = np.concatenate(tiles, axis=1).astype(FP16NP)

    bias_cols = []
    for l in range(L):
        for nm in ("be1", "be2", "bc1", "bv1", "bn1", "bn2"):
            bias_cols.append(np.tile(np.asarray(inp[nm][l]).reshape(-1), 2))
        for nm in ("bv2", "bc2"):
            bias_cols.append(
                np.full(128, float(np.asarray(inp[nm][l]).reshape(-1)[0]),
                        np.float32))
    bias_cols.append(np.tile(inp["emb_b"], 2))
    biaspack = np.stack(bias_cols, axis=1).astype(np.float32)
    return wpack, None, biaspack, None


def _arrange_inputs(obs_slice):
    """Per-core obs [4096, 20] -> invT fp16 [128,2048], locvelG f32 [128,128]."""
    obs3 = np.asarray(obs_slice, np.float32).reshape(G_CORE, N_AGENTS, INV + 4)
    invT = np.zeros((128, NNODE), np.float32)
    inv = obs3[:, :, 0:INV]                      # [g, i, f]
    invT[0:INV, :] = inv[0:64].transpose(2, 0, 1).reshape(INV, NNODE)
    invT[64:64 + INV, :] = inv[64:128].transpose(2, 0, 1).reshape(INV, NNODE)
    locvelG = np.ascontiguousarray(
        obs3[:, :, INV:INV + 4].reshape(G_CORE, 4 * N_AGENTS))
    return invT.astype(FP16NP), locvelG


def _unarrange_output(outP):
    """outP [128, 64] (g, (i,c)) -> [4096, 2]."""
    return np.asarray(outP, np.float32).reshape(NODES_CORE, 2)


# ----------------------------------------------------------------------------
# Device kernel builder
# ----------------------------------------------------------------------------

def build(scale0, scale1, mean0, mean1):
    import concourse.bacc as bacc
    import concourse.tile as tile
    import concourse.mybir as mybir
    from contextlib import ExitStack

    F32 = mybir.dt.float32
    FP16 = mybir.dt.float16
    AT = mybir.AluOpType
    ACTF = mybir.ActivationFunctionType
    AX = mybir.AxisListType.X

    nc = bacc.Bacc("TRN2", target_bir_lowering=False, debug=False)

    invT_d = nc.dram_tensor("invT", [128, NNODE], FP16, kind="ExternalInput")
    locvel_d = nc.dram_tensor("locvel", [128, 4 * N_AGENTS], F32,
                              kind="ExternalInput")
    NW = 1 + 12 * L
    wpack_d = nc.dram_tensor("wpack", [128, NW * 128], FP16,
                             kind="ExternalInput")
    NBIAS = 8 * L + 1
    bias_d = nc.dram_tensor("biaspack", [128, NBIAS], F32,
                            kind="ExternalInput")
    out_d = nc.dram_tensor("out", [128, 2 * N_AGENTS], F32,
                           kind="ExternalOutput")

    widx = {"emb": 0}
    _wi = 1
    for l in range(L):
        for nm in WNAMES:
            widx[f"{nm}{l}"] = _wi
            _wi += 1
    bidx = {}
    _bi = 0
    for l in range(L):
        for nm in BNAMES:
            bidx[f"{nm}{l}"] = _bi
            _bi += 1
    bidx["embb"] = _bi

    with tile.TileContext(nc) as tc, ExitStack() as ctx:
        st = ctx.enter_context(tc.tile_pool(name="static", bufs=1))
        p1 = ctx.enter_context(tc.tile_pool(name="p1", bufs=2, space="PSUM"))
        p2 = ctx.enter_context(tc.tile_pool(name="p2", bufs=1, space="PSUM"))
        p3 = ctx.enter_context(tc.tile_pool(name="p3", bufs=1, space="PSUM"))
        p4 = ctx.enter_context(tc.tile_pool(name="p4", bufs=1, space="PSUM"))
        em1 = ctx.enter_context(tc.tile_pool(name="em1", bufs=2))
        em2 = ctx.enter_context(tc.tile_pool(name="em2", bufs=3))
        ec1 = ctx.enter_context(tc.tile_pool(name="ec1", bufs=2))
        esb = ctx.enter_context(tc.tile_pool(name="esb", bufs=2))
        ersc = ctx.enter_context(tc.tile_pool(name="ersc", bufs=5))

        # ---- static loads ----
        wsb = st.tile([128, NW * 128], FP16)
        nc.sync.dma_start(wsb[:], wpack_d.ap())
        bsb = st.tile([128, NBIAS], F32)
        nc.sync.dma_start(bsb[:], bias_d.ap())
        invT = st.tile([128, NNODE], FP16)
        nc.sync.dma_start(invT[:], invT_d.ap())
        locvel = st.tile([128, 4 * N_AGENTS], F32)
        nc.sync.dma_start(locvel[:], locvel_d.ap())

        def W(name):
            return wsb[:, widx[name] * 128:(widx[name] + 1) * 128]

        def Bia(name):
            return bsb[:, bidx[name]:bidx[name] + 1]

        # ---- persistent state ----
        hA = st.tile([128, NNODE], FP16)
        hB = st.tile([128, NNODE], FP16)
        magg = st.tile([128, NNODE], FP16)
        mdiag = st.tile([128, NNODE], FP16)
        hv1 = st.tile([128, NNODE], FP16)
        phirep = st.tile([128, NNODE], F32)
        phiG = st.tile([128, N_AGENTS], F32)
        locx = st.tile([128, N_AGENTS], F32)
        locy = st.tile([128, N_AGENTS], F32)
        velx = st.tile([128, N_AGENTS], F32)
        vely = st.tile([128, N_AGENTS], F32)
        tvx = st.tile([128, N_AGENTS], F32)
        tvy = st.tile([128, N_AGENTS], F32)
        aggx = st.tile([128, N_AGENTS], FP16)
        aggy = st.tile([128, N_AGENTS], FP16)
        rad = st.tile([128, NE], FP16)
        dx = st.tile([128, NE], FP16)
        dy = st.tile([128, NE], FP16)
        sq = st.tile([128, NE], FP16)
        tm = st.tile([128, NE], FP16)
        um = st.tile([128, NE], FP16)
        wt = st.tile([128, NE], FP16)
        wt2 = st.tile([128, NE], FP16)
        smat = st.tile([128, NE], FP16)
        ea = st.tile([128, NE], FP16)
        outP = st.tile([128, 2 * N_AGENTS], F32)

        def heat(n=14):
            hp = p1.tile([128, 512], F32, tag="ps1")
            for _ in range(n):
                nc.tensor.matmul(hp[:], W("emb"), invT[:, 0:512],
                                 start=True, stop=True)

        heat()

        # ---- embedding: h0 = inv @ emb_W + emb_b ----
        for c in range(2):
            sl = slice(c * NE, (c + 1) * NE)
            pse = p2.tile([128, NE], F32, tag="ps2")
            for ih in range(2):
                osl = slice(ih * 512, (ih + 1) * 512)
                isl = slice(c * NE + ih * 512, c * NE + (ih + 1) * 512)
                nc.tensor.matmul(pse[:, osl], W("emb"), invT[:, isl],
                                 start=True, stop=True)
            with nc.allow_low_precision(reason="fp16 h"):
                nc.vector.tensor_scalar_add(hA[:, sl], pse[:], Bia("embb"))

        # ---- initial geometry (graph-major) ----
        lv = locvel[:].rearrange("p (i c) -> p i c", c=4)
        nc.vector.tensor_copy(locx[:], lv[:, :, 0])
        nc.vector.tensor_copy(locy[:], lv[:, :, 1])
        nc.vector.tensor_copy(velx[:], lv[:, :, 2])
        nc.vector.tensor_copy(vely[:], lv[:, :, 3])

        def rad_chain(first):
            """dx, dy, rad from locx/locy (fp16 out; DVE + GpSimd)."""
            with nc.allow_low_precision(reason="fp16 geometry"):
                for (lP, dT, eng) in ((locx, dx, nc.vector),
                                      (locy, dy, nc.vector)):
                    bi = lP[:].unsqueeze(2).broadcast_to(
                        [128, N_AGENTS, N_AGENTS])
                    bj = lP[:].unsqueeze(1).broadcast_to(
                        [128, N_AGENTS, N_AGENTS])
                    eng.tensor_tensor(
                        dT[:].rearrange("p (i j) -> p i j", j=N_AGENTS),
                        bi, bj, op=AT.subtract)
                nc.vector.tensor_tensor(wt2[:], dy[:], dy[:], op=AT.mult)
                nc.vector.tensor_tensor(rad[:], dx[:], dx[:], op=AT.mult)
                nc.vector.tensor_tensor(rad[:], rad[:], wt2[:], op=AT.add)
                if first:
                    nc.vector.tensor_copy(ea[:], rad[:])

        rad_chain(first=True)

        def prefetch_rsc(k):
            rsc = ersc.tile([4, NE], FP16, tag="rsc")
            nc.sync.dma_start(rsc[0:3:2, :], rad[k:k + 65:64, :])
            nc.sync.dma_start(rsc[1:4:2, :], ea[k:k + 65:64, :])
            return rsc

        for l in range(L):
            h = hA if l % 2 == 0 else hB
            h_next = hB if l % 2 == 0 else hA

            # ---- node phase: phi = silu(h@Wv1+bv1)@Wv2 + bv2 ----
            psv_q = [None] * 2
            for c in range(3):
                if c >= 1:
                    cc = c - 1
                    sl = slice(cc * NE, (cc + 1) * NE)
                    with nc.allow_low_precision(reason="fp16 hv1"):
                        for ih in range(2):
                            isl = slice(cc * NE + ih * 512,
                                        cc * NE + (ih + 1) * 512)
                            nc.scalar.activation(hv1[:, isl],
                                                 psv_q[cc][ih][:],
                                                 ACTF.Silu,
                                                 bias=Bia(f"bv1{l}"))
                    psv2 = p2.tile([128, NE], F32, tag="ps2")
                    for ih in range(2):
                        osl = slice(ih * 512, (ih + 1) * 512)
                        isl = slice(cc * NE + ih * 512, cc * NE + (ih + 1) * 512)
                        nc.tensor.matmul(psv2[:, osl], W(f"Wv2{l}"),
                                         hv1[:, isl], start=True, stop=True)
                    nc.vector.tensor_scalar_add(phirep[:, sl], psv2[:],
                                                Bia(f"bv2{l}"))
                    psv_q[cc] = None
                if c < 2:
                    sl = slice(c * NE, (c + 1) * NE)
                    pv = []
                    for ih in range(2):
                        psv = p1.tile([128, 512], F32, tag="ps1")
                        isl = slice(c * NE + ih * 512, c * NE + (ih + 1) * 512)
                        nc.tensor.matmul(psv[:], W(f"Wv1{l}"),
                                         h[:, isl], start=True, stop=True)
                        pv.append(psv)
                    psv_q[c] = pv
            # phi -> graph-major [g, i] (128 x 128B descriptors)
            nc.sync.dma_start(phiG[:, :], phirep[0:65:64, :])

            # ---- tm = 1/(1+sqrt(rad)) (overlaps edge phase on DVE) ----
            nc.scalar.activation(sq[:], rad[:], ACTF.Sqrt)
            with nc.allow_low_precision(reason="fp16 tm"):
                nc.vector.tensor_scalar_add(sq[:], sq[:], 1.0)
                nc.vector.reciprocal(tm[:], sq[:])

            # ---- edge phase: software-pipelined over 64 units ----
            rsc_q = [None] * NU
            ps1_q = [None] * NU
            ps2_q = [None] * NU
            ps3_q = [None] * NU
            ps4_q = [None] * NU
            m1_q = [None] * NU
            m_q = [None] * NU

            rsc_q[0] = prefetch_rsc(0)
            rsc_q[1] = prefetch_rsc(1)
            rsc_q[2] = prefetch_rsc(2)

            # Emission order per iteration: each PSUM pool's consumer comes
            # before that pool's next producer (bufs=1 slot recycling).
            for t in range(NU + 4):
                if 4 <= t:
                    # S8: evacuate ps4(t-4) -> SBUF; smat rows {k, 64+k}
                    k = t - 4
                    ssb = esb.tile([128, NE], FP16, tag="ssb")
                    with nc.allow_low_precision(reason="fp16 s"):
                        nc.vector.tensor_copy(ssb[:], ps4_q[k][:])
                    nc.sync.dma_start(smat[k:k + 65:64, :], ssb[0:65:64, :])
                    ps4_q[k] = None
                    rsc_q[k] = None
                if 3 <= t and t - 3 < NU:
                    # S6: ACT c1(t-3) frees p3; S7: ps4(t-3) fills p4
                    k = t - 3
                    c1 = ec1.tile([128, NE], FP16, tag="c1")
                    nc.scalar.activation(c1[:], ps3_q[k][:],
                                         ACTF.Silu, bias=Bia(f"bc1{l}"))
                    ps = p4.tile([128, NE], F32, tag="ps4")
                    for ih in range(2):
                        osl = slice(ih * 512, (ih + 1) * 512)
                        nc.tensor.matmul(ps[:, osl], W(f"Wc2{l}"),
                                         c1[:, osl], start=True, stop=True)
                    ps4_q[k] = ps
                    ps3_q[k] = None
                    m_q[k] = None
                if 2 <= t and t - 2 < NU:
                    # S4: ACT m(t-2) frees p2; S5: ps3(t-2) fills p3;
                    # magg/mdiag on GpSimd
                    k = t - 2
                    nb = k * 32
                    m_u = em2.tile([128, NE], FP16, tag="m")
                    nc.scalar.activation(m_u[:], ps2_q[k][:],
                                         ACTF.Silu, bias=Bia(f"be2{l}"))
                    ps = p3.tile([128, NE], F32, tag="ps3")
                    for ih in range(2):
                        osl = slice(ih * 512, (ih + 1) * 512)
                        nc.tensor.matmul(ps[:, osl], W(f"Wc1{l}"),
                                         m_u[:, osl], start=True, stop=True)
                    with nc.allow_low_precision(reason="fp16 magg"):
                        nc.vector.tensor_reduce(
                            magg[:, nb:nb + 32],
                            m_u[:].rearrange("p (i j) -> p i j", j=32),
                            axis=AX, op=AT.add)
                        nc.gpsimd.tensor_copy(mdiag[:, nb:nb + 32],
                                              m_u[:, 0:NE:33])
                    m_q[k] = m_u
                    ps3_q[k] = ps
                    ps2_q[k] = None
                if 1 <= t <= NU:
                    # S2: ACT m1(t-1) frees p1 halves; S3: ps2(t-1) fills p2
                    k = t - 1
                    m1 = em1.tile([128, NE], FP16, tag="m1")
                    ps = p2.tile([128, NE], F32, tag="ps2")
                    for ih in range(2):
                        osl = slice(ih * 512, (ih + 1) * 512)
                        nc.scalar.activation(m1[:, osl], ps1_q[k][ih][:],
                                             ACTF.Silu, bias=Bia(f"be1{l}"))
                        nc.tensor.matmul(ps[:, osl], W(f"We2{l}"),
                                         m1[:, osl], start=True, stop=True)
                    ps2_q[k] = ps
                    ps1_q[k] = None
                    m1_q[k] = None
                if t < NU:
                    # S1: ps1(t) = Wi@h_i + Wj@h_j + Wsc@[r, ea]
                    # (two [128,512] half tiles, 2-deep rotation in 2 banks)
                    k = t
                    nb = k * 32
                    hj = h[:, nb:nb + 32]
                    hj_bc = hj.unsqueeze(1).broadcast_to([128, 16, 32])
                    pa = []
                    for ih in range(2):
                        osl = slice(ih * 512, (ih + 1) * 512)
                        ps = p1.tile([128, 512], F32, tag="ps1")
                        hi = h[:, nb + ih * 16:nb + ih * 16 + 16]
                        hi_bc = hi.unsqueeze(2).broadcast_to([128, 16, 32])
                        nc.tensor.matmul(ps[:], W(f"Wi{l}"), hi_bc,
                                         start=True, stop=False)
                        nc.tensor.matmul(ps[:], W(f"Wj{l}"), hj_bc,
                                         start=False, stop=False)
                        nc.tensor.matmul(ps[:], W(f"Wsc{l}")[0:4, :],
                                         rsc_q[k][:, osl],
                                         start=False, stop=True)
                        pa.append(ps)
                    ps1_q[k] = pa
                if t + 3 < NU:
                    rsc_q[t + 3] = prefetch_rsc(t + 3)

            # ---- h update: h' = h + silu([h|magg]@Wn1+bn1)@Wn2 + bn2 ----
            psh_q = [None] * 2
            psh2_q = [None] * 2
            for c in range(4):
                if 2 <= c:
                    cc = c - 2
                    sl = slice(cc * NE, (cc + 1) * NE)
                    with nc.allow_low_precision(reason="fp16 h"):
                        nc.vector.scalar_tensor_tensor(
                            h_next[:, sl], psh2_q[cc][:], Bia(f"bn2{l}"),
                            h[:, sl], op0=AT.add, op1=AT.add)
                    psh2_q[cc] = None
                if 1 <= c <= 2:
                    cc = c - 1
                    sl = slice(cc * NE, (cc + 1) * NE)
                    hn1 = em1.tile([128, NE], FP16, tag="m1")
                    nc.scalar.activation(hn1[:], psh_q[cc][:], ACTF.Silu,
                                         bias=Bia(f"bn1{l}"))
                    psh2 = p4.tile([128, NE], F32, tag="ps4")
                    for ih in range(2):
                        osl = slice(ih * 512, (ih + 1) * 512)
                        nc.tensor.matmul(psh2[:, osl], W(f"Wn2{l}"),
                                         hn1[:, osl], start=True, stop=True)
                    psh2_q[cc] = psh2
                    psh_q[cc] = None
                if c < 2:
                    sl = slice(c * NE, (c + 1) * NE)
                    psh = p3.tile([128, NE], F32, tag="ps3")
                    for ih in range(2):
                        osl = slice(ih * 512, (ih + 1) * 512)
                        isl = slice(c * NE + ih * 512, c * NE + (ih + 1) * 512)
                        nc.tensor.matmul(psh[:, osl], W(f"Wn1t{l}"),
                                         h[:, isl], start=True, stop=False)
                        nc.tensor.matmul(psh[:, osl], W(f"Wn1b{l}"),
                                         magg[:, isl], start=False, stop=False)
                        nc.tensor.matmul(psh[:, osl], W(f"Wn1d{l}"),
                                         mdiag[:, isl], start=False, stop=True)
                    psh_q[c] = psh

            # ---- matrix phase: u, agg, vel/loc update (DVE x, GpSimd y) --
            with nc.allow_low_precision(reason="fp16 matrix"):
                nc.vector.scalar_tensor_tensor(um[:], smat[:], Bia(f"bc2{l}"),
                                               tm[:], op0=AT.add, op1=AT.mult)
                nc.vector.tensor_tensor(wt2[:], um[:], dy[:], op=AT.mult)
                nc.vector.tensor_tensor(wt[:], um[:], dx[:], op=AT.mult)
                for (lP, vP, ag, tv, w_) in (
                        (locx, velx, aggx, tvx, wt),
                        (locy, vely, aggy, tvy, wt2)):
                    nc.vector.tensor_reduce(
                        ag[:], w_[:].rearrange("p (i j) -> p i j", j=32),
                        axis=AX, op=AT.add)
                    nc.vector.tensor_tensor(tv[:], phiG[:], vP[:], op=AT.mult)
                    nc.vector.scalar_tensor_tensor(vP[:], ag[:], 1.0 / DEG,
                                                   tv[:], op0=AT.mult,
                                                   op1=AT.add)
                    nc.vector.tensor_tensor(lP[:], lP[:], vP[:], op=AT.add)
            if l < L - 1:
                rad_chain(first=False)

        # ---- output: outP[g, (i,c)] = scale * vel + mean ----
        ov = outP[:].rearrange("p (i c) -> p i c", c=2)
        nc.vector.tensor_scalar(ov[:, :, 0], velx[:], scale0, mean0,
                                op0=AT.mult, op1=AT.add)
        nc.vector.tensor_scalar(ov[:, :, 1], vely[:], scale1, mean1,
                                op0=AT.mult, op1=AT.add)
        nc.sync.dma_start(out_d.ap(), outP[:])

    nc.compile()
    return nc


# ----------------------------------------------------------------------------
# Entry point
# ----------------------------------------------------------------------------

def kernel(**inputs):
    import concourse.mybir  # noqa: F401  (ensure env importable)
    from concourse.bass_utils import run_bass_kernel_spmd

    inp = {k: np.asarray(v) for k, v in inputs.items()}
    obs = inp["obs"].astype(np.float32)
    scale = np.asarray(inp["scale"], np.float32)
    mean = np.asarray(inp["mean"], np.float32)

    key = (float(scale[0]), float(scale[1]), float(mean[0]), float(mean[1]))
    if key not in _BUILD_CACHE:
        _BUILD_CACHE[key] = build(*key)
    nc = _BUILD_CACHE[key]

    wpack, _, biaspack, _ = _pack_weights(inp)
    in_maps = []
    for c in range(NCORES):
        invT, locvel = _arrange_inputs(obs[c * NODES_CORE:(c + 1) * NODES_CORE])
        in_maps.append({"invT": invT, "locvel": locvel,
                        "wpack": wpack, "biaspack": biaspack})
    res = run_bass_kernel_spmd(nc, in_maps, list(range(NCORES)))
    outs = [_unarrange_output(res.results[c]["out"]) for c in range(NCORES)]
    return np.concatenate(outs, axis=0)
